# revision 10
# baseline (speedup 1.0000x reference)
"""Multi-head attention forward, sharded head-parallel across 8 NeuronCores.

Per core c (heads 2c, 2c+1):
  qT/kT/vT = (x @ W{q,k,v}_c.T).T        computed as W.T-tiled matmuls vs xT
  scoresT  = kT_chunk.T @ qT             [k-pos partitions, q-pos free]
  probsT   = exp(scoresT) (*causal mask via affine_select)
  av+rowsum: out.T = [v | 1].T @ probsT  (ones column yields softmax denom)
  normalize by PE-broadcast reciprocal, then out_projT partial
Host: sum the 8 partial [1024, 4096] outputs, transpose, add bias.
"""
import sys

sys.path.insert(0, "/opt/trn_rl_repo")

import numpy as np

B, S, D = 2, 2048, 1024
H, HD = 16, 64
NCORES = 8
SEC = 128           # output dims per core per section (2 heads * 64)
BS = B * S          # 4096
NT = BS // 512      # 8 seq tiles of 512
EC = D // 128       # 8 embed chunks
QT = S // 512       # 4 q-tiles per (b,h)
KC = S // 128       # 16 k-chunks per (b,h)

_cache = {}


def _build(mask_mode):
    import concourse.bass as bass
    import concourse.tile as tile
    from concourse import bacc, mybir

    f32 = mybir.dt.float32
    f32r = mybir.dt.float32r
    Exp = mybir.ActivationFunctionType.Exp

    nc = bacc.Bacc("TRN2", target_bir_lowering=False, debug=False,
                   num_devices=NCORES)

    xT = nc.dram_tensor("xT", [D, BS], f32r, kind="ExternalInput")
    wqkvT = nc.dram_tensor("wqkvT", [D, 3 * SEC], f32r, kind="ExternalInput")
    woT = nc.dram_tensor("woT", [SEC, D], f32r, kind="ExternalInput")
    # consts: [:, 0:64] = eye(64) on both partition halves; [:, 64:160] = 1.0
    consts = nc.dram_tensor("consts", [128, 160], f32r, kind="ExternalInput")
    if mask_mode == "general":
        maskT = nc.dram_tensor("maskT", [S, S], f32, kind="ExternalInput")
    out_pT = nc.dram_tensor("out_pT", [D, BS], f32, kind="ExternalOutput")

    with tile.TileContext(nc) as tc:
        with (
            nc.allow_low_precision(reason="fp32r passthrough transpose"),
            tc.tile_pool(name="singles", bufs=1) as singles,
            tc.tile_pool(name="qkv", bufs=1) as qkv,
            tc.tile_pool(name="xp", bufs=6) as xp,
            tc.tile_pool(name="v1p", bufs=2) as v1p,
            tc.tile_pool(name="pp", bufs=3) as pp,
            tc.tile_pool(name="np_", bufs=2) as np_,
            tc.tile_pool(name="fo", bufs=3) as fo,
        ):
            w_sb = singles.tile([128, EC, 3 * SEC], f32r)
            nc.sync.dma_start(
                out=w_sb[:], in_=wqkvT.rearrange("(ec p) c -> p ec c", p=128))
            woT_sb = singles.tile([128, D], f32r)
            nc.sync.dma_start(out=woT_sb[:], in_=woT[:])
            ident64 = singles.tile([128, 64], f32r)
            nc.sync.dma_start(out=ident64[:], in_=consts[:, 0:64])
            ones1 = singles.tile([1, 64], f32r)
            nc.sync.dma_start(out=ones1[:], in_=consts[0:1, 64:128])

            qT = qkv.tile([128, BS], f32r)
            kT = qkv.tile([128, BS], f32r)
            vT = qkv.tile([128, BS], f32r)
            ocat = qkv.tile([128, BS], f32r)

            # ---- stage A: qkvT projections ----
            with tc.tile_pool(name="psA", bufs=2, space="PSUM") as psA:
                for n in range(NT):
                    pq = psA.tile([128, 512], f32, tag="pq")
                    pk = psA.tile([128, 512], f32, tag="pk")
                    pv = psA.tile([128, 512], f32, tag="pv")
                    for ec in range(EC):
                        xt = xp.tile([128, 512], f32r, tag="xt")
                        nc.sync.dma_start(
                            out=xt[:],
                            in_=xT[128 * ec:128 * (ec + 1),
                                   512 * n:512 * (n + 1)])
                        st, sp = ec == 0, ec == EC - 1
                        nc.tensor.matmul(pq[:], w_sb[:, ec, 0:128],
                                         xt[:], start=st, stop=sp)
                        nc.tensor.matmul(pk[:], w_sb[:, ec, 128:256],
                                         xt[:], start=st, stop=sp)
                        nc.tensor.matmul(pv[:], w_sb[:, ec, 256:384],
                                         xt[:], start=st, stop=sp)
                    sl = slice(512 * n, 512 * (n + 1))
                    nc.any.tensor_copy(qT[:, sl], pq[:])
                    nc.any.tensor_copy(kT[:, sl], pk[:])
                    nc.any.tensor_copy(vT[:, sl], pv[:])

            # ---- stage B: attention per (b, local head) ----
            with (
                tc.tile_pool(name="psS", bufs=2, space="PSUM") as psS,
                tc.tile_pool(name="psO", bufs=2, space="PSUM") as psO,
                tc.tile_pool(name="psT", bufs=2, space="PSUM") as psT,
            ):
                for b in range(B):
                    for lh in range(2):
                        hsl = slice(64 * lh, 64 * (lh + 1))
                        base = S * b
                        # v1 = [v | 1] in natural [seq, hd] layout
                        v1 = v1p.tile([128, KC, HD + 1], f32r, tag="v1")
                        nc.sync.dma_start(out=v1[:, :, HD],
                                          in_=consts[:, 64:64 + KC])
                        for i in range(KC):
                            pt = psT.tile([128, 64], f32r, tag="tb")
                            nc.tensor.transpose(
                                pt[:],
                                vT[hsl, base + 128 * i:base + 128 * (i + 1)],
                                ident64[hsl, :])
                            nc.any.tensor_copy(v1[:, i, 0:HD], pt[:])
                        for t in range(QT):
                            qsl = slice(base + 512 * t, base + 512 * (t + 1))
                            po = psO.tile([HD + 1, 512], f32, tag="po")
                            njc = 4 * t + 4 if mask_mode == "causal" else KC
                            ng = njc // 2
                            for g in range(ng):
                                ps = psS.tile([128, 1024], f32, tag="ps")
                                for jj in range(2):
                                    j = 2 * g + jj
                                    nc.tensor.matmul(
                                        ps[:, 512 * jj:512 * (jj + 1)],
                                        kT[hsl, base + 128 * j:
                                             base + 128 * (j + 1)],
                                        qT[hsl, qsl],
                                        start=True, stop=True)
                                pr = pp.tile([128, 1024], f32r, tag="pr")
                                nc.scalar.activation(pr[:], ps[:], Exp)
                                for jj in range(2):
                                    j = 2 * g + jj
                                    sub = pr[:, 512 * jj:512 * (jj + 1)]
                                    if mask_mode == "causal" and j >= 4 * t:
                                        jm = j - 4 * t
                                        nc.gpsimd.affine_select(
                                            out=sub, in_=sub,
                                            compare_op=mybir.AluOpType.is_ge,
                                            fill=0.0, base=-128 * jm,
                                            channel_multiplier=-1,
                                            pattern=[[1, 512]])
                                    elif mask_mode == "general":
                                        msk = xp.tile([128, 512], f32,
                                                      tag="msk")
                                        nc.sync.dma_start(
                                            out=msk[:],
                                            in_=maskT[128 * j:128 * (j + 1),
                                                      512 * t:512 * (t + 1)])
                                        nc.vector.tensor_mul(sub, sub, msk[:])
                                for jj in range(2):
                                    j = 2 * g + jj
                                    nc.tensor.matmul(
                                        po[:],
                                        v1[:, j, :],
                                        pr[:, 512 * jj:512 * (jj + 1)],
                                        start=(g == 0 and jj == 0),
                                        stop=(g == ng - 1 and jj == 1))
                            # normalize: ocat[hsl, qsl] = po[0:64] * (1/l)
                            rc = np_.tile([1, 512], f32r, tag="rc")
                            nc.vector.reciprocal(rc[:], po[HD:HD + 1, :])
                            pb = psT.tile([64, 512], f32, tag="tb")
                            nc.tensor.matmul(pb[:], ones1[:], rc[:],
                                             start=True, stop=True)
                            bs_ = np_.tile([64, 512], f32, tag="bs")
                            nc.any.tensor_copy(bs_[:], pb[:])
                            nc.vector.tensor_mul(ocat[hsl, qsl],
                                                 po[0:HD, :], bs_[:])

            # ---- stage C: out_projT partial ----
            with tc.tile_pool(name="psF", bufs=4, space="PSUM") as psF:
                for oc in range(EC):
                    osl = slice(128 * oc, 128 * (oc + 1))
                    for n in range(NT):
                        ssl = slice(512 * n, 512 * (n + 1))
                        pf = psF.tile([128, 512], f32, tag="pf")
                        nc.tensor.matmul(pf[:], woT_sb[:, osl],
                                         ocat[:, ssl],
                                         start=True, stop=True)
                        ft = fo.tile([128, 512], f32, tag="ft")
                        nc.any.tensor_copy(ft[:], pf[:])
                        nc.sync.dma_start(out=out_pT[osl, ssl], in_=ft[:])

    nc.compile()
    return nc


def _classify_mask(mask):
    m = np.asarray(mask).reshape(S, S) != 0
    if m.all():
        return "none", None
    if np.array_equal(m, np.tril(np.ones((S, S), bool))):
        return "causal", None
    return "general", m.T.astype(np.float32)


def _ensure_ntff_hook():
    """Register antenv.axon_hooks with a ctypes NTFF profile hook if the
    container image lacks it (mirrors trn_agent_boot's registration)."""
    import types
    try:
        from antenv.axon_hooks import get_axon_ntff_profile_hook  # noqa: F401
        return
    except ImportError:
        pass
    import contextlib
    import ctypes

    hook = None
    so_path = "/opt/axon/libaxon_pjrt.so"
    try:
        lib = ctypes.CDLL(so_path)
        if hasattr(lib, "axon_start_nrt_profile"):
            lib.axon_start_nrt_profile.argtypes = [
                ctypes.POINTER(ctypes.c_int64), ctypes.c_size_t]
            lib.axon_start_nrt_profile.restype = ctypes.c_int64
            lib.axon_stop_nrt_profile.argtypes = [ctypes.c_char_p]
            lib.axon_stop_nrt_profile.restype = ctypes.c_int64

            @contextlib.contextmanager
            def _hook(output_dir, device_ids):
                import jax
                jax.devices()
                if device_ids:
                    ids = (ctypes.c_int64 * len(device_ids))(*device_ids)
                    rc = lib.axon_start_nrt_profile(ids, len(device_ids))
                else:
                    rc = lib.axon_start_nrt_profile(None, 0)
                if rc != 0:
                    raise RuntimeError(f"axon_start_nrt_profile rc={rc}")
                try:
                    yield
                finally:
                    n = lib.axon_stop_nrt_profile(str(output_dir).encode())
                    print(f"profile: {n} file(s) written to {output_dir}",
                          flush=True)

            hook = _hook
    except OSError:
        pass

    mod = types.ModuleType("antenv.axon_hooks")
    _h = [hook]
    mod.get_axon_ntff_profile_hook = lambda: _h[0]

    def _set(h):
        _h[0] = h

    mod.set_axon_ntff_profile_hook = _set
    sys.modules["antenv.axon_hooks"] = mod
    try:
        import antenv
        antenv.axon_hooks = mod
    except ImportError:
        pass


def kernel(key, query, value, mask, W_qkv, W_out, b_out):
    from concourse.bass_utils import run_bass_kernel_spmd
    import os

    mask_mode, maskT = _classify_mask(mask)
    if mask_mode not in _cache:
        _cache[mask_mode] = _build(mask_mode)
    nc = _cache[mask_mode]

    x = np.ascontiguousarray(
        np.asarray(query, np.float32).reshape(BS, D))
    xT = np.ascontiguousarray(x.T)
    W_qkv = np.asarray(W_qkv, np.float32)
    W_out = np.asarray(W_out, np.float32)

    in_maps = []
    for c in range(NCORES):
        sl = slice(SEC * c, SEC * (c + 1))
        wq = W_qkv[sl, :].T * np.float32(HD ** -0.5)
        wk = W_qkv[D + SEC * c:D + SEC * (c + 1), :].T
        wv = W_qkv[2 * D + SEC * c:2 * D + SEC * (c + 1), :].T
        consts = np.zeros((128, 160), np.float32)
        consts[0:64, 0:64] = np.eye(64, dtype=np.float32)
        consts[64:128, 0:64] = np.eye(64, dtype=np.float32)
        consts[:, 64:160] = 1.0
        m = {
            "xT": xT,
            "consts": consts,
            "wqkvT": np.ascontiguousarray(
                np.concatenate([wq, wk, wv], axis=1, dtype=np.float32)),
            "woT": np.ascontiguousarray(W_out[:, sl].T),
        }
        if mask_mode == "general":
            m["maskT"] = maskT
        in_maps.append(m)

    trace = bool(int(os.environ.get("KERNEL_TRACE", "0")))
    if trace:
        _ensure_ntff_hook()
        try:
            res = run_bass_kernel_spmd(nc, in_maps,
                                       core_ids=list(range(NCORES)),
                                       trace=True)
        except Exception as e:
            print(f"traced run failed ({e!r}); retrying untraced",
                  flush=True)
            res = run_bass_kernel_spmd(nc, in_maps,
                                       core_ids=list(range(NCORES)))
        print(f"HW exec time: {res.exec_time_ns} ns", flush=True)
        kernel.last_exec_ns = res.exec_time_ns
        kernel.last_results = res
    else:
        res = run_bass_kernel_spmd(nc, in_maps, core_ids=list(range(NCORES)))

    acc = res.results[0]["out_pT"].astype(np.float32)
    for c in range(1, NCORES):
        acc = acc + res.results[c]["out_pT"]
    out = acc.T.reshape(B, S, D) + np.asarray(b_out, np.float32)
    return out.astype(np.float32)


# revision 12
# speedup vs baseline: 1.1827x; 1.1827x over previous
"""Multi-head attention forward, sharded head-parallel across 8 NeuronCores.

Per core c (heads 2c, 2c+1):
  qT/kT/vT = (x @ W{q,k,v}_c.T).T        computed as W.T-tiled matmuls vs xT
  scoresT  = kT_chunk.T @ qT             [k-pos partitions, q-pos free]
  probsT   = exp(scoresT) (*causal mask via affine_select)
  av+rowsum: out.T = [v | 1].T @ probsT  (ones column yields softmax denom)
  normalize by PE-broadcast reciprocal, then out_projT partial
Host: sum the 8 partial [1024, 4096] outputs, transpose, add bias.
"""
import sys

sys.path.insert(0, "/opt/trn_rl_repo")

import ml_dtypes
import numpy as np

BF16 = ml_dtypes.bfloat16

B, S, D = 2, 2048, 1024
H, HD = 16, 64
NCORES = 8
SEC = 128           # output dims per core per section (2 heads * 64)
BS = B * S          # 4096
NT = BS // 512      # 8 seq tiles of 512
EC = D // 128       # 8 embed chunks
QT = S // 512       # 4 q-tiles per (b,h)
KC = S // 128       # 16 k-chunks per (b,h)

_cache = {}


def _build(mask_mode):
    import concourse.bass as bass
    import concourse.tile as tile
    from concourse import bacc, mybir

    f32 = mybir.dt.float32
    bf16 = mybir.dt.bfloat16
    Exp = mybir.ActivationFunctionType.Exp

    nc = bacc.Bacc("TRN2", target_bir_lowering=False, debug=False,
                   num_devices=NCORES)

    xT = nc.dram_tensor("xT", [D, BS], bf16, kind="ExternalInput")
    wqkvT = nc.dram_tensor("wqkvT", [D, 3 * SEC], bf16, kind="ExternalInput")
    woT = nc.dram_tensor("woT", [SEC, D], bf16, kind="ExternalInput")
    # consts: [:, 0:64] = eye(64) on both partition halves; [:, 64:160] = 1.0
    consts = nc.dram_tensor("consts", [128, 160], bf16, kind="ExternalInput")
    if mask_mode == "general":
        maskT = nc.dram_tensor("maskT", [S, S], bf16, kind="ExternalInput")
    out_pT = nc.dram_tensor("out_pT", [D, BS], f32, kind="ExternalOutput")

    with tile.TileContext(nc) as tc:
        with (
            nc.allow_low_precision(reason="fp32r passthrough transpose"),
            tc.tile_pool(name="singles", bufs=1) as singles,
            tc.tile_pool(name="qkv", bufs=1) as qkv,
            tc.tile_pool(name="xp", bufs=6) as xp,
            tc.tile_pool(name="v1p", bufs=2) as v1p,
            tc.tile_pool(name="pp", bufs=3) as pp,
            tc.tile_pool(name="np_", bufs=2) as np_,
            tc.tile_pool(name="fo", bufs=3) as fo,
        ):
            w_sb = singles.tile([128, EC, 3 * SEC], bf16)
            nc.sync.dma_start(
                out=w_sb[:], in_=wqkvT.rearrange("(ec p) c -> p ec c", p=128))
            woT_sb = singles.tile([128, D], bf16)
            nc.sync.dma_start(out=woT_sb[:], in_=woT[:])
            ident64 = singles.tile([128, 64], bf16)
            nc.sync.dma_start(out=ident64[:], in_=consts[:, 0:64])
            ones1 = singles.tile([1, 64], f32)
            nc.vector.memset(ones1[:], 1.0)

            qT = qkv.tile([128, BS], bf16)
            kT = qkv.tile([128, BS], bf16)
            vT = qkv.tile([128, BS], bf16)
            ocat = qkv.tile([128, BS], bf16)

            # ---- stage A: qkvT projections ----
            with tc.tile_pool(name="psA", bufs=2, space="PSUM") as psA:
                for n in range(NT):
                    pq = psA.tile([128, 512], f32, tag="pq")
                    pk = psA.tile([128, 512], f32, tag="pk")
                    pv = psA.tile([128, 512], f32, tag="pv")
                    for ec in range(EC):
                        xt = xp.tile([128, 512], bf16, tag="xt")
                        nc.sync.dma_start(
                            out=xt[:],
                            in_=xT[128 * ec:128 * (ec + 1),
                                   512 * n:512 * (n + 1)])
                        st, sp = ec == 0, ec == EC - 1
                        nc.tensor.matmul(pq[:], w_sb[:, ec, 0:128],
                                         xt[:], start=st, stop=sp)
                        nc.tensor.matmul(pk[:], w_sb[:, ec, 128:256],
                                         xt[:], start=st, stop=sp)
                        nc.tensor.matmul(pv[:], w_sb[:, ec, 256:384],
                                         xt[:], start=st, stop=sp)
                    sl = slice(512 * n, 512 * (n + 1))
                    nc.vector.tensor_copy(qT[:, sl], pq[:])
                    nc.vector.tensor_copy(kT[:, sl], pk[:])
                    nc.vector.tensor_copy(vT[:, sl], pv[:])

            # ---- stage B: attention per (b, local head) ----
            with (
                tc.tile_pool(name="psS", bufs=2, space="PSUM") as psS,
                tc.tile_pool(name="psO", bufs=2, space="PSUM") as psO,
                tc.tile_pool(name="psT", bufs=2, space="PSUM") as psT,
            ):
                for b in range(B):
                    for lh in range(2):
                        hsl = slice(64 * lh, 64 * (lh + 1))
                        base = S * b
                        # v1 = [v | 1] in natural [seq, hd] layout
                        v1 = v1p.tile([128, KC, HD + 1], bf16, tag="v1")
                        nc.sync.dma_start(out=v1[:, :, HD],
                                          in_=consts[:, 64:64 + KC])
                        for i in range(KC):
                            pt = psT.tile([128, 64], bf16, tag="tb")
                            nc.tensor.transpose(
                                pt[:],
                                vT[hsl, base + 128 * i:base + 128 * (i + 1)],
                                ident64[hsl, :])
                            nc.vector.tensor_copy(v1[:, i, 0:HD], pt[:])
                        for t in range(QT):
                            qsl = slice(base + 512 * t, base + 512 * (t + 1))
                            po = psO.tile([HD + 1, 512], f32, tag="po")
                            njc = 4 * t + 4 if mask_mode == "causal" else KC
                            ng = njc // 2
                            for g in range(ng):
                                ps = psS.tile([128, 1024], f32, tag="ps")
                                for jj in range(2):
                                    j = 2 * g + jj
                                    nc.tensor.matmul(
                                        ps[:, 512 * jj:512 * (jj + 1)],
                                        kT[hsl, base + 128 * j:
                                             base + 128 * (j + 1)],
                                        qT[hsl, qsl],
                                        start=True, stop=True)
                                pr = pp.tile([128, 1024], bf16, tag="pr")
                                nc.scalar.activation(pr[:], ps[:], Exp)
                                for jj in range(2):
                                    j = 2 * g + jj
                                    sub = pr[:, 512 * jj:512 * (jj + 1)]
                                    if mask_mode == "causal" and j >= 4 * t:
                                        jm = j - 4 * t
                                        nc.gpsimd.affine_select(
                                            out=sub, in_=sub,
                                            compare_op=mybir.AluOpType.is_ge,
                                            fill=0.0, base=-128 * jm,
                                            channel_multiplier=-1,
                                            pattern=[[1, 512]])
                                    elif mask_mode == "general":
                                        msk = xp.tile([128, 512], bf16,
                                                      tag="msk")
                                        nc.sync.dma_start(
                                            out=msk[:],
                                            in_=maskT[128 * j:128 * (j + 1),
                                                      512 * t:512 * (t + 1)])
                                        nc.vector.tensor_mul(sub, sub, msk[:])
                                for jj in range(2):
                                    j = 2 * g + jj
                                    nc.tensor.matmul(
                                        po[:],
                                        v1[:, j, :],
                                        pr[:, 512 * jj:512 * (jj + 1)],
                                        start=(g == 0 and jj == 0),
                                        stop=(g == ng - 1 and jj == 1))
                            # normalize: ocat[hsl, qsl] = po[0:64] * (1/l)
                            lc = np_.tile([1, 512], f32, tag="lc")
                            nc.vector.tensor_copy(lc[:], po[HD:HD + 1, :])
                            rc = np_.tile([1, 512], f32, tag="rc")
                            nc.vector.reciprocal_approx_fast(rc[:], lc[:])
                            pb = psT.tile([64, 512], f32, tag="tb")
                            nc.tensor.matmul(pb[:], ones1[:], rc[:],
                                             start=True, stop=True)
                            bs_ = np_.tile([64, 512], bf16, tag="bs")
                            nc.scalar.copy(bs_[:], pb[:])
                            nc.vector.tensor_mul(ocat[hsl, qsl],
                                                 po[0:HD, :], bs_[:])

            # ---- stage C: out_projT partial ----
            with tc.tile_pool(name="psF", bufs=4, space="PSUM") as psF:
                for oc in range(EC):
                    osl = slice(128 * oc, 128 * (oc + 1))
                    for n in range(NT):
                        ssl = slice(512 * n, 512 * (n + 1))
                        pf = psF.tile([128, 512], f32, tag="pf")
                        nc.tensor.matmul(pf[:], woT_sb[:, osl],
                                         ocat[:, ssl],
                                         start=True, stop=True)
                        ft = fo.tile([128, 512], f32, tag="ft")
                        nc.vector.tensor_copy(ft[:], pf[:])
                        nc.sync.dma_start(out=out_pT[osl, ssl], in_=ft[:])

    nc.compile()
    return nc


def _classify_mask(mask):
    m = np.asarray(mask).reshape(S, S) != 0
    if m.all():
        return "none", None
    if np.array_equal(m, np.tril(np.ones((S, S), bool))):
        return "causal", None
    return "general", m.T.astype(np.float32)


def _ensure_ntff_hook():
    """Register antenv.axon_hooks with a ctypes NTFF profile hook if the
    container image lacks it (mirrors trn_agent_boot's registration)."""
    import types
    try:
        from antenv.axon_hooks import get_axon_ntff_profile_hook  # noqa: F401
        return
    except ImportError:
        pass
    import contextlib
    import ctypes

    hook = None
    so_path = "/opt/axon/libaxon_pjrt.so"
    try:
        lib = ctypes.CDLL(so_path)
        if hasattr(lib, "axon_start_nrt_profile"):
            lib.axon_start_nrt_profile.argtypes = [
                ctypes.POINTER(ctypes.c_int64), ctypes.c_size_t]
            lib.axon_start_nrt_profile.restype = ctypes.c_int64
            lib.axon_stop_nrt_profile.argtypes = [ctypes.c_char_p]
            lib.axon_stop_nrt_profile.restype = ctypes.c_int64

            @contextlib.contextmanager
            def _hook(output_dir, device_ids):
                import jax
                jax.devices()
                if device_ids:
                    ids = (ctypes.c_int64 * len(device_ids))(*device_ids)
                    rc = lib.axon_start_nrt_profile(ids, len(device_ids))
                else:
                    rc = lib.axon_start_nrt_profile(None, 0)
                if rc != 0:
                    raise RuntimeError(f"axon_start_nrt_profile rc={rc}")
                try:
                    yield
                finally:
                    n = lib.axon_stop_nrt_profile(str(output_dir).encode())
                    print(f"profile: {n} file(s) written to {output_dir}",
                          flush=True)

            hook = _hook
    except OSError:
        pass

    mod = types.ModuleType("antenv.axon_hooks")
    _h = [hook]
    mod.get_axon_ntff_profile_hook = lambda: _h[0]

    def _set(h):
        _h[0] = h

    mod.set_axon_ntff_profile_hook = _set
    sys.modules["antenv.axon_hooks"] = mod
    try:
        import antenv
        antenv.axon_hooks = mod
    except ImportError:
        pass


def kernel(key, query, value, mask, W_qkv, W_out, b_out):
    from concourse.bass_utils import run_bass_kernel_spmd
    import os

    mask_mode, maskT = _classify_mask(mask)
    if mask_mode not in _cache:
        _cache[mask_mode] = _build(mask_mode)
    nc = _cache[mask_mode]

    x = np.ascontiguousarray(
        np.asarray(query, np.float32).reshape(BS, D))
    xT_bf = np.ascontiguousarray(x.T).astype(BF16)
    W_qkv = np.asarray(W_qkv, np.float32)
    W_out = np.asarray(W_out, np.float32)

    in_maps = []
    for c in range(NCORES):
        sl = slice(SEC * c, SEC * (c + 1))
        wq = W_qkv[sl, :].T * np.float32(HD ** -0.5)
        wk = W_qkv[D + SEC * c:D + SEC * (c + 1), :].T
        wv = W_qkv[2 * D + SEC * c:2 * D + SEC * (c + 1), :].T
        consts = np.zeros((128, 160), BF16)
        consts[0:64, 0:64] = np.eye(64, dtype=BF16)
        consts[64:128, 0:64] = np.eye(64, dtype=BF16)
        consts[:, 64:160] = 1.0
        m = {
            "xT": xT_bf,
            "consts": consts,
            "wqkvT": np.ascontiguousarray(np.concatenate(
                [wq, wk, wv], axis=1, dtype=np.float32)).astype(BF16),
            "woT": np.ascontiguousarray(W_out[:, sl].T).astype(BF16),
        }
        if mask_mode == "general":
            m["maskT"] = maskT.astype(BF16)
        in_maps.append(m)

    trace = bool(int(os.environ.get("KERNEL_TRACE", "0")))
    if trace:
        _ensure_ntff_hook()
        try:
            res = run_bass_kernel_spmd(nc, in_maps,
                                       core_ids=list(range(NCORES)),
                                       trace=True)
        except Exception as e:
            print(f"traced run failed ({e!r}); retrying untraced",
                  flush=True)
            res = run_bass_kernel_spmd(nc, in_maps,
                                       core_ids=list(range(NCORES)))
        print(f"HW exec time: {res.exec_time_ns} ns", flush=True)
        kernel.last_exec_ns = res.exec_time_ns
        kernel.last_results = res
    else:
        res = run_bass_kernel_spmd(nc, in_maps, core_ids=list(range(NCORES)))

    acc = res.results[0]["out_pT"].astype(np.float32)
    for c in range(1, NCORES):
        acc = acc + res.results[c]["out_pT"]
    out = acc.T.reshape(B, S, D) + np.asarray(b_out, np.float32)
    return out.astype(np.float32)
